# revision 19
# baseline (speedup 1.0000x reference)
"""Trainium2 Bass kernel for nn_EpisodicMemory (modularity + conductance).

Per batch element (N=2048 rows, D=512 dims):
    S = rep @ rep.T            (never materialized)
    S' = S / max(||S_row||, 1e-12)
    communities = contiguous runs given by cumsum(boundaries)
    mod  = (sum_c W_c - sum_c D_c^2/total) / total,  total = sum_c D_c
    cond = mean_c (D_c - W_c)/(W_c + D_c + 1e-10)

Formulation (one f32r pass for each big matmul, fp32 everywhere it
matters for the ill-conditioned conductance denominators):
    G = rep^T rep  (f32r matmul, fp32 PSUM accum)
    H = rep @ G    (f32r), ssq_i = <rep_i, H_i>, rnorm_i = 1/max(sqrt,eps)
    rowsum_i = <rep_i, u> (exact fp32 PE matvec), deg_i = rnorm_i*rowsum_i
    b-scan: segmented prefix sums of repT        -> R_c at segment ends
    a-scan: segmented prefix sums of rnorm*repT  -> V_c at segment ends
    W_c = <V_c, R_c> (fp32 ones-matvec partition reduce of a*b)
    D_c = segmented sum of deg (16,128 scan with cross-partition carries)

Phases B and C are fused: per 512-column chunk j, the H tiles complete,
ssq -> rnorm -> broadcast -> va -> a-scan -> a*b -> W reduce follow
immediately, pipelined across engines.

Sharding: data-parallel over the batch axis, one batch element per core,
8 NeuronCores. Full inputs in, full (2, 8) output out.
"""
import sys
if '/opt/trn_rl_repo' not in sys.path:
    sys.path.insert(0, '/opt/trn_rl_repo')

import numpy as np

N = 2048
D = 512
NT = N // 128          # 16 row tiles
ND = D // 128          # 4 partition chunks of repT
NG = 4                 # tile groups (4 tiles = 512 rows each)
NJ = N // 512          # free chunks of 512
EPS_NORM = 1e-12
EPS_COND = 1e-10

_COMPILED = None


def _build():
    import concourse.bacc as bacc
    import concourse.tile as tile
    from concourse import mybir
    from concourse.masks import make_identity

    f32 = mybir.dt.float32
    f32r = mybir.dt.float32r
    bf16 = mybir.dt.bfloat16
    i32 = mybir.dt.int32
    Alu = mybir.AluOpType
    Act = mybir.ActivationFunctionType

    nc = bacc.Bacc("TRN2", target_bir_lowering=False, debug=False)
    rep_d = nc.dram_tensor("rep", [N, D], f32, kind="ExternalInput")
    bnd_d = nc.dram_tensor("bnd", [N], i32, kind="ExternalInput")
    out_d = nc.dram_tensor("out", [1, 2], f32, kind="ExternalOutput")

    # rep4_d[g, k][p, d] = rep[g*512 + k*128 + p, d]
    rep4_d = rep_d.rearrange("(g k p) d -> g k p d", g=NG, k=4, p=128)
    b_row_d = bnd_d.rearrange("(a f) -> a f", a=1)

    with tile.TileContext(nc) as tc:
        with (
            tc.tile_pool(name="big", bufs=1) as big,
            tc.tile_pool(name="rows", bufs=3) as rows,
            tc.tile_pool(name="small", bufs=1) as small,
            tc.tile_pool(name="pgm", bufs=1, space="PSUM") as pgm,   # G accum (4 tags)
            tc.tile_pool(name="pwk", bufs=3, space="PSUM") as pwk,   # transposes/H
            tc.tile_pool(name="psm", bufs=1, space="PSUM") as psm,   # small
        ):
            # ---------- constants ----------
            ident = small.tile([128, 128], f32)
            make_identity(nc, ident[:])
            ones_col = small.tile([128, 1], f32)
            nc.vector.memset(ones_col[:], 1.0)
            ones_row = small.tile([1, 128], f32)
            nc.vector.memset(ones_row[:], 1.0)

            # ---------- load inputs (split over HWDGE queues) ----------
            rep4 = []
            for g in range(NG):
                rt = big.tile([128, 4 * D], f32, tag=f"blkA{g}")
                rep4.append(rt)
            dma_engs = [nc.sync, nc.scalar, nc.gpsimd]
            for g in range(NG):
                for k in range(4):
                    eng = dma_engs[(g * 4 + k) % 3]
                    eng.dma_start(rep4[g][:, k*D:(k+1)*D], rep4_d[g, k])
            b_row = rows.tile([1, N], i32, tag="rowbuf")
            nc.sync.dma_start(b_row[:], b_row_d[:])

            # ---------- masks ----------
            bf_row = rows.tile([1, N], f32, tag="rowbuf")
            nc.scalar.activation(bf_row[:], b_row[:], Act.Copy)
            m_row = rows.tile([1, N], bf16, tag="rowbuf")  # 0 at starts
            nc.scalar.activation(m_row[:], bf_row[:], Act.Copy,
                                 bias=1.0, scale=-1.0)
            l_row = rows.tile([1, N], f32, tag="rowbuf")   # 1 at ends
            nc.vector.memset(l_row[:, N-1:N], 1.0)
            nc.scalar.activation(l_row[:, 0:N-1], bf_row[:, 1:N], Act.Copy)

            m_td = small.tile([16, 128], bf16)
            nc.sync.dma_start(m_td[:], m_row.rearrange("a (p f) -> a p f", p=16))
            l_td = small.tile([16, 128], f32)
            nc.sync.dma_start(l_td[:], l_row.rearrange("a (p f) -> a p f", p=16))

            m_bc = big.tile([128, N], bf16, tag="m_bc")
            nc.gpsimd.partition_broadcast(m_bc[:], m_row[:])

            # products of m along each (16,128) row: carry propagation mask
            Pm = small.tile([16, 128], f32)
            nc.vector.tensor_tensor_scan(out=Pm[:], data0=m_td[:],
                                         data1=m_td[:], initial=1.0,
                                         op0=Alu.mult, op1=Alu.bypass)
            bch_ps = psm.tile([1, 16], f32, tag="sm")
            nc.tensor.transpose(bch_ps[:], Pm[:, 127:128], ident[:16, :16])
            bch_row = small.tile([1, 16], f32)
            nc.vector.tensor_copy(bch_row[:], bch_ps[:])

            # ---------- phase A: transpose, f32r round, G accum ----------
            repT = []          # exact fp32 transpose [128(d), 2048(i)]
            for dc in range(ND):
                rT = big.tile([128, N], f32, tag=f"blkC{dc}")
                repT.append(rT)
            repT_r = []        # f32r-rounded transpose
            for dc in range(ND):
                rT = big.tile([128, N], f32, tag=f"blkD{dc}")
                repT_r.append(rT)
            bsc = []           # b-scan outputs (prefix segment sums of repT)
            for dc in range(ND):
                bt = big.tile([128, N], f32, tag=f"blkE{dc}")
                bsc.append(bt)

            g_ps = [pgm.tile([128, D], f32, tag=f"g{mc}", name=f"g_ps{mc}")
                    for mc in range(4)]

            for g in range(NG):
                # round this group to f32r for the G matmuls (transient)
                rr = big.tile([128, 4 * D], f32, tag="repr", bufs=2,
                              name=f"rep_r{g}")
                nc.scalar.activation(rr[:].bitcast(f32r), rep4[g][:], Act.Copy)
                # transposes first (no ACT dependency) -> hide the rounding
                for dc in range(ND):
                    tp_ps = pwk.tile([128, D], f32, tag="wk")
                    for k in range(4):
                        nc.tensor.transpose(
                            tp_ps[:, k*128:(k+1)*128],
                            rep4[g][:, k*D+dc*128:k*D+(dc+1)*128], ident[:])
                    nc.scalar.copy(repT[dc][:, g*D:(g+1)*D], tp_ps[:])
                    nc.vector.tensor_copy(
                        repT_r[dc][:, g*D:(g+1)*D].bitcast(f32r),
                        repT[dc][:, g*D:(g+1)*D])
                for k in range(4):
                    t = g * 4 + k
                    # G[mc,:] += rep_tile[:,mc128]^T @ rep_tile   (f32r)
                    for mc in range(4):
                        nc.tensor.matmul(
                            g_ps[mc][:],
                            rr[:, k*D+mc*128:k*D+(mc+1)*128].bitcast(f32r),
                            rr[:, k*D:(k+1)*D].bitcast(f32r),
                            start=(t == 0), stop=(t == NT - 1))
                # b-scan chunk for this group (chained via initial)
                for dc in range(ND):
                    ini = 0.0 if g == 0 else bsc[dc][:, g*D-1:g*D]
                    nc.vector.tensor_tensor_scan(
                        out=bsc[dc][:, g*D:(g+1)*D],
                        data0=m_bc[:, g*D:(g+1)*D],
                        data1=repT[dc][:, g*D:(g+1)*D],
                        initial=ini, op0=Alu.mult, op1=Alu.add)

            # G -> SBUF as f32r (rounded during evacuation)
            G_r = big.tile([128, 4 * D], f32, tag="G")
            for mc in range(4):
                nc.scalar.activation(G_r[:, mc*D:(mc+1)*D].bitcast(f32r),
                                     g_ps[mc][:], Act.Copy)

            # ---------- u = column sums of rep (exact, ACT accum) ------
            u_cols16 = small.tile([128, 16], f32)
            for dc in range(ND):
                junk = big.tile([128, 4 * D], f32, tag="repr", bufs=2,
                                name=f"ujunk{dc}")
                for j in range(NJ):
                    nc.scalar.activation(
                        junk[:, j*D:(j+1)*D], repT[dc][:, j*D:(j+1)*D],
                        Act.Copy, accum_out=u_cols16[:, dc*4+j:dc*4+j+1])
            u_cols = small.tile([128, ND], f32)
            for dc in range(ND):
                nc.vector.tensor_reduce(out=u_cols[:, dc:dc+1],
                                        in_=u_cols16[:, dc*4:(dc+1)*4],
                                        axis=mybir.AxisListType.X, op=Alu.add)

            # ---------- phase B+C fused, chunked by j ----------
            ssq_cols = small.tile([128, NT], f32)
            rowsum_row = rows.tile([1, N], f32, tag="rowbuf")
            w_row = rows.tile([1, N], f32, tag="rowbuf")
            rn_td = small.tile([16, 128], f32)
            carry = [small.tile([128, 1], f32, name=f"carry{dc}")
                     for dc in range(ND)]
            rnbc = {}

            def chunk_head(j, prev_tail=None):
                # H tiles of this chunk (PE first in queue)
                h_tiles = []
                for k in range(4):
                    t = j * 4 + k
                    h_ps = pwk.tile([128, D], f32, tag="wk", name=f"h_ps{t}")
                    for dc in range(ND):
                        nc.tensor.matmul(
                            h_ps[:],
                            repT_r[dc][:, t*128:(t+1)*128].bitcast(f32r),
                            G_r[:, dc*D:(dc+1)*D].bitcast(f32r),
                            start=(dc == 0), stop=(dc == ND - 1))
                    h_tiles.append(h_ps)
                # ssq accumulation (DVE drains H psums as they complete)
                for k in range(4):
                    t = j * 4 + k
                    sc = big.tile([128, D], f32, tag="scr", name=f"ssq{t}")
                    nc.vector.scalar_tensor_tensor(
                        out=sc[:], in0=rep4[j][:, k*D:(k+1)*D],
                        scalar=0.0, in1=h_tiles[k][:],
                        op0=Alu.add, op1=Alu.mult,
                        accum_out=ssq_cols[:, t:t+1])

                # rnorm chunk j: sqrt/recip then exact PE broadcast
                tps = pwk.tile([4, 128], f32, tag="wk", name=f"tps{j}")
                nc.tensor.transpose(tps[:], ssq_cols[:, 4*j:4*(j+1)],
                                    ident[:])
                nrm4 = small.tile([4, 128], f32, tag="nrm4", bufs=2, name=f"nrm4_{j}")
                nc.scalar.activation(nrm4[:], tps[:], Act.Sqrt)
                nc.vector.tensor_scalar(out=nrm4[:], in0=nrm4[:],
                                        scalar1=EPS_NORM, scalar2=None,
                                        op0=Alu.max)
                rn4 = small.tile([4, 128], f32, tag="rnblk", bufs=2, name=f"rn4_{j}")
                nc.vector.reciprocal(rn4[:], nrm4[:])
                rnrow4 = small.tile([1, D], f32, tag="rnrow", bufs=2, name=f"rnrow{j}")
                nc.sync.dma_start(rnrow4[:], rn4[:])
                nc.scalar.dma_start(rn_td[4*j:4*(j+1), :], rn4[:])

                # rowsum chunk j (exact fp32 matvec u^T repT)
                rsp = psm.tile([1, D], f32, tag="sm", name=f"rsp{j}")
                for dc in range(ND):
                    nc.tensor.matmul(rsp[:], u_cols[:, dc:dc+1],
                                     repT[dc][:, j*D:(j+1)*D],
                                     start=(dc == 0), stop=(dc == ND - 1))
                nc.vector.tensor_copy(rowsum_row[:, j*D:(j+1)*D], rsp[:])

                # previous chunk's tail now (overlaps this chunk's rnorm)
                if prev_tail is not None:
                    prev_tail()

                bc_ps = pgm.tile([128, D], f32, tag=f"g{j}", name=f"rnbc{j}")
                nc.tensor.matmul(bc_ps[:], ones_row[:], rnrow4[:],
                                 start=True, stop=True)
                rnbc[j] = bc_ps

            def chunk_tail(j):
                # va, a-scan, a*b, W reduce for chunk j
                wp = psm.tile([1, D], f32, tag="sm", name=f"wp{j}")
                for dc in range(ND):
                    va = big.tile([128, D], f32, tag="vj", bufs=3,
                                  name=f"va{dc}_{j}")
                    nc.vector.tensor_tensor(out=va[:], in0=rnbc[j][:],
                                            in1=repT[dc][:, j*D:(j+1)*D],
                                            op=Alu.mult)
                    asc = big.tile([128, D], f32, tag="ascj", bufs=2,
                                   name=f"asc{dc}_{j}")
                    ini = 0.0 if j == 0 else carry[dc][:]
                    nc.vector.tensor_tensor_scan(
                        out=asc[:], data0=m_bc[:, j*D:(j+1)*D], data1=va[:],
                        initial=ini, op0=Alu.mult, op1=Alu.add)
                    if j < NJ - 1:
                        nc.vector.tensor_copy(carry[dc][:], asc[:, D-1:D])
                    ab = big.tile([128, D], f32, tag="vj", bufs=3,
                                  name=f"ab{dc}_{j}")
                    eng = nc.gpsimd if dc < 2 else nc.vector
                    eng.tensor_tensor(out=ab[:], in0=asc[:],
                                      in1=bsc[dc][:, j*D:(j+1)*D],
                                      op=Alu.mult)
                    nc.tensor.matmul(wp[:], ones_col[:], ab[:],
                                     start=(dc == 0), stop=(dc == ND - 1))
                nc.vector.tensor_copy(w_row[:, j*D:(j+1)*D], wp[:])

            # software pipeline: chunk j's tail overlaps chunk j+1's head
            chunk_head(0)
            for j in range(1, NJ):
                chunk_head(j, prev_tail=lambda jj=j-1: chunk_tail(jj))
            chunk_tail(NJ - 1)

            rs_td = small.tile([16, 128], f32)
            nc.sync.dma_start(rs_td[:],
                              rowsum_row.rearrange("a (p f) -> a p f", p=16))
            w_td = small.tile([16, 128], f32)
            nc.sync.dma_start(w_td[:], w_row.rearrange("a (p f) -> a p f", p=16))
            deg16 = small.tile([16, 128], f32, tag="e16", bufs=7, name="deg16")
            nc.vector.tensor_tensor(out=deg16[:], in0=rn_td[:], in1=rs_td[:],
                                    op=Alu.mult)

            # ---------- segmented sum of deg (16,128) with carries ------
            segD0 = small.tile([16, 128], f32, tag="e16", bufs=7, name="segD0")
            nc.vector.tensor_tensor_scan(out=segD0[:], data0=m_td[:],
                                         data1=deg16[:], initial=0.0,
                                         op0=Alu.mult, op1=Alu.add)

            aD_ps = psm.tile([1, 16], f32, tag="sm")
            nc.tensor.transpose(aD_ps[:], segD0[:, 127:128], ident[:16, :16])
            aD_row = small.tile([1, 16], f32)
            nc.vector.tensor_copy(aD_row[:], aD_ps[:])
            incl = small.tile([1, 16], f32)
            nc.vector.tensor_tensor_scan(out=incl[:], data0=bch_row[:],
                                         data1=aD_row[:], initial=0.0,
                                         op0=Alu.mult, op1=Alu.add)
            excl = small.tile([1, 16], f32)
            nc.vector.memset(excl[:, 0:1], 0.0)
            nc.vector.tensor_copy(excl[:, 1:16], incl[:, 0:15])
            cc_ps = psm.tile([16, 1], f32, tag="sm")
            nc.tensor.transpose(cc_ps[:], excl[:], ident[:1, :1])
            iD_col = small.tile([16, 1], f32)
            nc.vector.tensor_copy(iD_col[:], cc_ps[:])

            segD = small.tile([16, 128], f32, tag="e16", bufs=7, name="segD")
            nc.vector.scalar_tensor_tensor(
                out=segD[:], in0=Pm[:], scalar=iD_col[:], in1=segD0[:],
                op0=Alu.mult, op1=Alu.add)

            # ---------- final reductions ----------
            Dl = small.tile([16, 128], f32, tag="e16", bufs=7, name="Dl")
            nc.vector.tensor_tensor(out=Dl[:], in0=segD[:], in1=l_td[:],
                                    op=Alu.mult)
            Wl = small.tile([16, 128], f32, tag="e16", bufs=7, name="Wl")
            nc.vector.tensor_tensor(out=Wl[:], in0=w_td[:], in1=l_td[:],
                                    op=Alu.mult)

            acc5 = small.tile([16, 5], f32)
            scr16 = small.tile([16, 128], f32, tag="e16", bufs=7, name="scr16")
            nc.vector.tensor_scalar(out=scr16[:], in0=Wl[:], scalar1=1.0,
                                    scalar2=0.0, op0=Alu.mult, op1=Alu.add,
                                    accum_out=acc5[:, 0:1])
            nc.vector.scalar_tensor_tensor(
                out=scr16[:], in0=Dl[:], scalar=0.0, in1=Dl[:],
                op0=Alu.add, op1=Alu.mult, accum_out=acc5[:, 1:2])
            num = small.tile([16, 128], f32, tag="e16", bufs=7, name="num")
            nc.vector.tensor_tensor(out=num[:], in0=Dl[:], in1=Wl[:],
                                    op=Alu.subtract)
            den = small.tile([16, 128], f32, tag="e16", bufs=7, name="den")
            nc.vector.tensor_tensor(out=den[:], in0=Dl[:], in1=Wl[:],
                                    op=Alu.add)
            lz = small.tile([16, 128], f32, tag="e16", bufs=7, name="lz")
            nc.vector.tensor_scalar(out=lz[:], in0=l_td[:],
                                    scalar1=(EPS_COND - 1.0), scalar2=1.0,
                                    op0=Alu.mult, op1=Alu.add)
            nc.vector.tensor_tensor(out=den[:], in0=den[:], in1=lz[:],
                                    op=Alu.add)
            rden = small.tile([16, 128], f32, tag="e16", bufs=7, name="den")
            nc.vector.reciprocal(rden[:], den[:])
            nc.vector.scalar_tensor_tensor(
                out=scr16[:], in0=num[:], scalar=0.0, in1=rden[:],
                op0=Alu.add, op1=Alu.mult, accum_out=acc5[:, 2:3])
            nc.vector.tensor_scalar(out=scr16[:], in0=l_td[:], scalar1=1.0,
                                    scalar2=0.0, op0=Alu.mult, op1=Alu.add,
                                    accum_out=acc5[:, 3:4])
            nc.vector.tensor_scalar(out=scr16[:], in0=Dl[:], scalar1=1.0,
                                    scalar2=0.0, op0=Alu.mult, op1=Alu.add,
                                    accum_out=acc5[:, 4:5])

            a5_ps = psm.tile([5, 16], f32, tag="sm")
            nc.tensor.transpose(a5_ps[:], acc5[:], ident[:16, :16])
            a5T = small.tile([5, 16], f32)
            nc.vector.tensor_copy(a5T[:], a5_ps[:])
            sums5 = small.tile([5, 1], f32)
            nc.vector.tensor_reduce(out=sums5[:], in_=a5T[:],
                                    axis=mybir.AxisListType.X, op=Alu.add)
            s5_ps = psm.tile([1, 5], f32, tag="sm")
            nc.tensor.transpose(s5_ps[:], sums5[:], ident[:5, :5])
            srow = small.tile([1, 5], f32)
            nc.vector.tensor_copy(srow[:], s5_ps[:])

            # srow = [W_sum, Dsq, cond_sum, n_comms, total]
            rtot = small.tile([1, 1], f32)
            nc.vector.reciprocal(rtot[:], srow[:, 4:5])
            t1 = small.tile([1, 1], f32)
            nc.vector.tensor_tensor(out=t1[:], in0=srow[:, 1:2], in1=rtot[:],
                                    op=Alu.mult)
            modn = small.tile([1, 1], f32)
            nc.vector.tensor_tensor(out=modn[:], in0=srow[:, 0:1], in1=t1[:],
                                    op=Alu.subtract)
            out_s = small.tile([1, 2], f32)
            nc.vector.tensor_tensor(out=out_s[:, 0:1], in0=modn[:],
                                    in1=rtot[:], op=Alu.mult)
            ncc = small.tile([1, 1], f32)
            nc.vector.tensor_scalar(out=ncc[:], in0=srow[:, 3:4], scalar1=1.0,
                                    scalar2=None, op0=Alu.max)
            rncc = small.tile([1, 1], f32)
            nc.vector.reciprocal(rncc[:], ncc[:])
            nc.vector.tensor_tensor(out=out_s[:, 1:2], in0=srow[:, 2:3],
                                    in1=rncc[:], op=Alu.mult)

            nc.sync.dma_start(out_d[:], out_s[:])

    nc.compile()
    return nc


def _get_compiled():
    global _COMPILED
    if _COMPILED is None:
        _COMPILED = _build()
    return _COMPILED


def _run(representations, boundaries, trace=False):
    from concourse.bass_utils import run_bass_kernel_spmd
    nc = _get_compiled()
    B = representations.shape[0]
    in_maps = [
        {"rep": np.ascontiguousarray(representations[i], dtype=np.float32),
         "bnd": np.ascontiguousarray(boundaries[i], dtype=np.int32)}
        for i in range(B)
    ]
    res = run_bass_kernel_spmd(nc, in_maps, list(range(B)), trace=trace)
    out = np.stack([res.results[i]["out"][0] for i in range(B)], axis=1)
    return out.astype(np.float32), res


def kernel(representations, boundaries):
    out, _ = _run(np.asarray(representations), np.asarray(boundaries))
    return out
